# revision 9
# baseline (speedup 1.0000x reference)
"""Trainium2 Bass kernel for nn_GunnarODE: neural CDE with hermite spline control.

Contract: kernel(**inputs) takes FULL unsharded inputs (ts, us, ys, W1, b1,
W2, b2, batch_size) and returns the FULL (B, L, Y) output. Internally shards
the batch across 8 NeuronCores (pure data parallel), runs a Bass/Tile kernel
per core, and reassembles.

Algorithm notes (derived from the reference):
  - x = concat([t, us]) with unit-spaced knots (ts is arange) => dt == 1.
  - Hermite backward-difference spline derivative at substep s_i = i/4 of
    interval k reduces to dXdt_i = alpha_i * slope_{k-1} + beta_i * slope_k
    with alpha_i = 1-4s+3s^2, beta_i = 4s-3s^2; the time channel has
    dXdt == 1.
  - Per Euler substep: h = tanh(z@W1.T+b1); vf = tanh(h@W2.T+b2) viewed as
    (Y=16, C=9); z += 0.25 * einsum(vf, dXdt).
  - On device everything is kept transposed (feature on partitions, batch on
    the free dim). The 144 vf rows are split into 128 "ctrl" rows
    (r=(c-1)*16+y for channels c=1..8) and 16 "time" rows (y*9).
  - All matmuls are fp32: the ODE amplifies per-step rounding ~1e5x, so
    reduced-precision matmuls (fp32r/bf16) fail the accuracy budget.

Performance structure (v4): four fp32 matmul passes over the batch per
substep (PE floor: fp32 = 4 cycles/moving-row):
  1. yva = W2a @ th       (128 ctrl pre-activations, column halves)
  2. yvb = W2b @ th       (16 time pre-activations, column halves)
  3. dXb = Abc_i @ u3     (spline derivative broadcast, column halves)
  4. hpre += (h*W1*Sel^T) @ tmp  (state update, column halves)
The dXb matmuls for substep t+1 are dependency-free (u3 is prefetched), so
they are placed at the two points where the state-update chain would
otherwise stall the tensor engine (before yva-first and before the
W1Sel pair): they keep the PE busy so it holds its ramped p-state.  The
time-channel contribution is folded into tmp rows 0..15 with an in-place
DVE add (those rows carry weight HSTEP*W1[:,y] in the W1Sel matmul).  The
half-column priority alternates each substep so the two half-chains average
their fast/slow queue positions.  Per interval the hpre snapshot is DMA'd
out and z = pinv(W1) @ hpre runs on the host.
"""
import sys
if '/opt/trn_rl_repo' not in sys.path:
    sys.path.insert(0, '/opt/trn_rl_repo')

import numpy as np

N_CORES = 8
L = 512
B_TOT = 4096
U = 8
Y = 16
H = 128
C = U + 1
NI = L - 1            # intervals
HSTEP = 0.25          # dt / SUBSTEPS with dt == 1
B_LOC = B_TOT // N_CORES  # 512

ALPHA = [1.0, 0.1875, -0.25, -0.3125]
BETA = [0.0, 0.8125, 1.25, 1.3125]

_BUILD_CACHE = {}


def _host_constants(W1, b1, W2, b2):
    """Precompute transposed/permuted constant matrices (host-side, free)."""
    rowmap = np.array([(r % 16) * 9 + (r // 16 + 1) for r in range(128)])
    cst = {}
    cst["W1T"] = np.ascontiguousarray(W1.T)                        # (16,128)
    cst["W2aT"] = np.ascontiguousarray(W2[rowmap, :].T)            # (128,128)
    cst["W2bT"] = np.ascontiguousarray(W2[np.arange(16) * 9, :].T)  # (128,16)
    cst["b1c"] = np.ascontiguousarray(b1[:, None])                 # (128,1)
    cst["b2c"] = np.ascontiguousarray(b2[rowmap][:, None])         # (128,1)
    cst["b2t"] = np.ascontiguousarray(b2[np.arange(16) * 9][:, None])  # (16,1)
    # spline eval: dXb[r] = -a*u3[c-1] + (a-b)*u3[8+c-1] + b*u3[16+c-1]
    abc = np.zeros((4, 24, 128), dtype=np.float32)
    for i in range(4):
        for r in range(128):
            c = r // 16 + 1
            abc[i, 0 * 8 + c - 1, r] = -ALPHA[i]
            abc[i, 1 * 8 + c - 1, r] = ALPHA[i] - BETA[i]
            abc[i, 2 * 8 + c - 1, r] = BETA[i]
    cst["Abc"] = abc                                               # (4,24,128)
    # state update matrix: hpre += (h*W1*Sel^T) @ tmp, [r, j] = h*W1[j, r%16]
    w1selt = np.zeros((128, 128), dtype=np.float32)
    for r in range(128):
        w1selt[r, :] = HSTEP * W1[:, r % 16]
    cst["W1SelT"] = w1selt
    return {k: v.astype(np.float32) for k, v in cst.items()}


def _build(n_intervals=NI):
    """Build + compile the Bass module (cached per interval count)."""
    key = n_intervals
    if key in _BUILD_CACHE:
        return _BUILD_CACHE[key]

    import concourse.bass as bass
    import concourse.bacc as bacc
    import concourse.tile as tile
    from concourse import mybir

    F32 = mybir.dt.float32
    TANH = mybir.ActivationFunctionType.Tanh
    MULT = mybir.AluOpType.mult
    ADD = mybir.AluOpType.add

    nc = bacc.Bacc("TRN2", target_bir_lowering=False, debug=False,
                   num_devices=N_CORES)

    d_us3 = nc.dram_tensor("us3", (n_intervals, 24, B_LOC), F32,
                           kind="ExternalInput")
    d_ys0 = nc.dram_tensor("ys0T", (16, B_LOC), F32, kind="ExternalInput")
    d_W1T = nc.dram_tensor("W1T", (16, 128), F32, kind="ExternalInput")
    d_W2aT = nc.dram_tensor("W2aT", (128, 128), F32, kind="ExternalInput")
    d_W2bT = nc.dram_tensor("W2bT", (128, 16), F32, kind="ExternalInput")
    d_b1 = nc.dram_tensor("b1c", (128, 1), F32, kind="ExternalInput")
    d_b2c = nc.dram_tensor("b2c", (128, 1), F32, kind="ExternalInput")
    d_b2t = nc.dram_tensor("b2t", (16, 1), F32, kind="ExternalInput")
    d_Abc = nc.dram_tensor("Abc", (4, 24, 128), F32, kind="ExternalInput")
    d_W1SelT = nc.dram_tensor("W1SelT", (128, 128), F32, kind="ExternalInput")
    d_hout = nc.dram_tensor("hout", (n_intervals, 128, B_LOC), F32,
                            kind="ExternalOutput")

    with tile.TileContext(nc) as tc:
        with (
            tc.tile_pool(name="consts", bufs=1) as consts,
            tc.tile_pool(name="work", bufs=2) as work,
            tc.tile_pool(name="u3p", bufs=4) as u3p,
            tc.tile_pool(name="hsp", bufs=2) as hsp,
            tc.tile_pool(name="ps1", bufs=1, space="PSUM") as ps1,
            tc.tile_pool(name="ps2", bufs=1, space="PSUM") as ps2,
            tc.tile_pool(name="psx", bufs=2, space="PSUM") as psx,
        ):
            W1T = consts.tile([16, 128], F32)
            W2aT = consts.tile([128, 128], F32)
            W2bT = consts.tile([128, 16], F32)
            b1c = consts.tile([128, 1], F32)
            b2c = consts.tile([128, 1], F32)
            b2t = consts.tile([16, 1], F32)
            W1SelT = consts.tile([128, 128], F32)
            Abc = [consts.tile([24, 128], F32, name=f"Abc{i}") for i in range(4)]
            nc.sync.dma_start(W1T[:], d_W1T.ap())
            nc.sync.dma_start(W2aT[:], d_W2aT.ap())
            nc.sync.dma_start(W2bT[:], d_W2bT.ap())
            nc.sync.dma_start(b1c[:], d_b1.ap())
            nc.sync.dma_start(b2c[:], d_b2c.ap())
            nc.sync.dma_start(b2t[:], d_b2t.ap())
            nc.sync.dma_start(W1SelT[:], d_W1SelT.ap())
            for i in range(4):
                nc.sync.dma_start(Abc[i][:], d_Abc.ap()[i])

            z0 = consts.tile([16, B_LOC], F32)
            nc.sync.dma_start(z0[:], d_ys0.ap())

            # hpre is THE state: a persistent PSUM accumulator holding W1 @ z.
            hpre = ps1.tile([128, B_LOC], F32, tag="hpre")
            nc.tensor.matmul(hpre[:], W1T[:], z0[:], start=True, stop=False,
                             skip_group_check=True)

            HB = B_LOC // 2
            COLS = [(0, HB), (HB, B_LOC)]
            u3s = {}

            def load_u3(k):
                if k < n_intervals:
                    t = u3p.tile([24, B_LOC], F32, tag="u3", name=f"u3_{k}")
                    nc.sync.dma_start(t[:], d_us3.ap()[k])
                    u3s[k] = t

            for k in range(3):
                load_u3(k)

            # dXb for substep g, one PSUM tile per half, computed one substep
            # ahead at the PE's two natural stall points.
            dxbs = {}

            def dxb_mm(g, h):
                if g >= 4 * n_intervals:
                    return
                k, i = divmod(g, 4)
                c0, c1 = COLS[h]
                t = psx.tile([128, HB], F32, tag=f"dxb{h}",
                             name=f"dxb{h}_{g}")
                nc.tensor.matmul(t[:], Abc[i][:], u3s[k][:, c0:c1],
                                 start=True, stop=True)
                dxbs[(g, h)] = t

            # substep 0 halves up front
            dxb_mm(0, 0)
            dxb_mm(0, 1)

            for k in range(n_intervals):
                load_u3(k + 3)
                for i in range(4):
                    g = 4 * k + i
                    # alternate which half leads so the half-chains average
                    # their fast/slow queue positions
                    ha, hb = (0, 1) if (g % 2 == 0) else (1, 0)
                    (a0, a1), (b0, b1_) = COLS[ha], COLS[hb]
                    dxa = dxbs.pop((g, ha))
                    dxb = dxbs.pop((g, hb))
                    th = work.tile([128, B_LOC], F32, tag="th")
                    yva = [ps2.tile([128, HB], F32, tag=f"yva{h}",
                                    name=f"yva{h}_{g}") for h in range(2)]
                    yvb = ps2.tile([16, B_LOC], F32, tag="yvb", name=f"yvb_{g}")
                    vfc = work.tile([128, B_LOC], F32, tag="vfc")
                    vft = work.tile([16, B_LOC], F32, tag="vft")
                    tmp = work.tile([128, B_LOC], F32, tag="tmp")

                    # PE queue: dxb(g+1,ha), yva_a, yvb_a, yva_b, yvb_b,
                    #           dxb(g+1,hb), W1Sel_a, W1Sel_b
                    dxb_mm(g + 1, ha)
                    nc.scalar.activation(th[:, a0:a1], hpre[:, a0:a1],
                                         TANH, bias=b1c[:])
                    nc.tensor.matmul(yva[ha][:], W2aT[:], th[:, a0:a1],
                                     start=True, stop=True)
                    nc.scalar.activation(th[:, b0:b1_], hpre[:, b0:b1_],
                                         TANH, bias=b1c[:])
                    nc.tensor.matmul(yvb[:, a0:a1], W2bT[:], th[:, a0:a1],
                                     start=True, stop=True)
                    nc.scalar.activation(vfc[:, a0:a1], yva[ha][:], TANH,
                                         bias=b2c[:])
                    nc.scalar.activation(vft[:, a0:a1], yvb[:, a0:a1],
                                         TANH, bias=b2t[:])
                    nc.vector.tensor_tensor(tmp[:, a0:a1], vfc[:, a0:a1],
                                            dxa[:], MULT)
                    # tmp rows 0..15 (channel 1) also carry the time channel:
                    # + vft adds HSTEP*W1@vft to hpre below.
                    nc.vector.tensor_tensor(tmp[:16, a0:a1], tmp[:16, a0:a1],
                                            vft[:, a0:a1], ADD)
                    nc.tensor.matmul(yva[hb][:], W2aT[:], th[:, b0:b1_],
                                     start=True, stop=True)
                    nc.tensor.matmul(yvb[:, b0:b1_], W2bT[:], th[:, b0:b1_],
                                     start=True, stop=True)
                    nc.scalar.activation(vfc[:, b0:b1_], yva[hb][:], TANH,
                                         bias=b2c[:])
                    nc.scalar.activation(vft[:, b0:b1_], yvb[:, b0:b1_],
                                         TANH, bias=b2t[:])
                    nc.vector.tensor_tensor(tmp[:, b0:b1_], vfc[:, b0:b1_],
                                            dxb[:], MULT)
                    nc.vector.tensor_tensor(tmp[:16, b0:b1_], tmp[:16, b0:b1_],
                                            vft[:, b0:b1_], ADD)
                    dxb_mm(g + 1, hb)
                    nc.tensor.matmul(hpre[:, a0:a1], W1SelT[:],
                                     tmp[:, a0:a1], start=False,
                                     stop=False, skip_group_check=True)
                    nc.tensor.matmul(hpre[:, b0:b1_], W1SelT[:],
                                     tmp[:, b0:b1_], start=False,
                                     stop=False, skip_group_check=True)
                # per-interval output: snapshot hpre in halves (each half as
                # soon as its last W1Sel lands); host recovers
                # z_{k+1} = pinv(W1) @ hpre.
                hps = hsp.tile([128, B_LOC], F32, tag="hps")
                ha = (4 * k + 3) % 2  # leading half of the last substep
                c0, c1 = COLS[ha]
                nc.vector.tensor_copy(hps[:, c0:c1], hpre[:, c0:c1])
                c0, c1 = COLS[1 - ha]
                nc.vector.tensor_copy(hps[:, c0:c1], hpre[:, c0:c1])
                nc.sync.dma_start(d_hout.ap()[k], hps[:])

    nc.compile()
    _BUILD_CACHE[key] = nc
    return nc


def _prep_core_inputs(us, ys, cst, core, n_intervals):
    b0 = core * B_LOC
    usc = np.ascontiguousarray(us[:, b0:b0 + B_LOC, :].transpose(0, 2, 1))  # (L,8,B)
    us_ext = np.concatenate([2.0 * usc[:1] - usc[1:2], usc], axis=0)  # (L+1,8,B)
    sw = np.lib.stride_tricks.sliding_window_view(us_ext, 3, axis=0)  # (L-1,8,B,3)
    us3 = np.ascontiguousarray(sw.transpose(0, 3, 1, 2).reshape(L - 1, 24, B_LOC))
    us3 = us3[:n_intervals].astype(np.float32)
    ys0T = np.ascontiguousarray(ys[0, b0:b0 + B_LOC, :].T).astype(np.float32)
    m = {"us3": us3, "ys0T": ys0T}
    m.update(cst)
    return m


def kernel(ts, us, ys, W1, b1, W2, b2, batch_size=None, n_intervals=NI):
    from concourse.bass_utils import run_bass_kernel_spmd

    us = np.asarray(us, dtype=np.float32)
    ys = np.asarray(ys, dtype=np.float32)
    W1 = np.asarray(W1, np.float32)
    cst = _host_constants(W1, np.asarray(b1, np.float32),
                          np.asarray(W2, np.float32), np.asarray(b2, np.float32))
    nc = _build(n_intervals)
    in_maps = [_prep_core_inputs(us, ys, cst, c, n_intervals) for c in range(N_CORES)]
    res = run_bass_kernel_spmd(nc, in_maps, core_ids=list(range(N_CORES)))
    # output reconstruction: z = pinv(W1) @ hpre  (W1 is 128x16, cond ~2)
    R = np.linalg.pinv(W1.astype(np.float64)).astype(np.float32)   # (16,128)
    out = np.empty((B_TOT, n_intervals + 1, Y), dtype=np.float32)
    out[:, 0, :] = ys[0]
    for c in range(N_CORES):
        b0 = c * B_LOC
        hout = res.results[c]["hout"]                # (NI, 128, B_LOC)
        z = np.tensordot(R, hout, axes=(1, 1))       # (16, NI, B_LOC)
        out[b0:b0 + B_LOC, 1:, :] = z.transpose(2, 1, 0)
    kernel._last_results = res
    return out


# revision 13
# speedup vs baseline: 1.0001x; 1.0001x over previous
"""Trainium2 Bass kernel for nn_GunnarODE: neural CDE with hermite spline control.

Contract: kernel(**inputs) takes FULL unsharded inputs (ts, us, ys, W1, b1,
W2, b2, batch_size) and returns the FULL (B, L, Y) output. Internally shards
the batch across 8 NeuronCores (pure data parallel), runs a Bass/Tile kernel
per core, and reassembles.

Algorithm notes (derived from the reference):
  - x = concat([t, us]) with unit-spaced knots (ts is arange) => dt == 1.
  - Hermite backward-difference spline derivative at substep s_i = i/4 of
    interval k reduces to dXdt_i = alpha_i * slope_{k-1} + beta_i * slope_k
    with alpha_i = 1-4s+3s^2, beta_i = 4s-3s^2; the time channel has
    dXdt == 1.
  - Per Euler substep: h = tanh(z@W1.T+b1); vf = tanh(h@W2.T+b2) viewed as
    (Y=16, C=9); z += 0.25 * einsum(vf, dXdt).
  - On device everything is kept transposed (feature on partitions, batch on
    the free dim). The 144 vf rows are split into 128 "ctrl" rows
    (r=(c-1)*16+y for channels c=1..8) and 16 "time" rows (y*9).
  - All matmuls are fp32: the ODE amplifies per-step rounding ~1e5x, so
    reduced-precision matmuls (fp32r/bf16) fail the accuracy budget.

Performance structure (v4): four fp32 matmul passes over the batch per
substep (PE floor: fp32 = 4 cycles/moving-row):
  1. yva = W2a @ th       (128 ctrl pre-activations, column halves)
  2. yvb = W2b @ th       (16 time pre-activations, column halves)
  3. dXb = Abc_i @ u3     (spline derivative broadcast, column halves)
  4. hpre += (h*W1*Sel^T) @ tmp  (state update, column halves)
The dXb matmuls for substep t+1 are dependency-free (u3 is prefetched), so
they are placed at the two points where the state-update chain would
otherwise stall the tensor engine (before yva-first and before the
W1Sel pair): they keep the PE busy so it holds its ramped p-state.  The
time-channel contribution is folded into tmp rows 0..15 with an in-place
DVE add (those rows carry weight HSTEP*W1[:,y] in the W1Sel matmul).  The
half-column priority alternates each substep so the two half-chains average
their fast/slow queue positions.  Per interval the hpre snapshot is DMA'd
out and z = pinv(W1) @ hpre runs on the host.
"""
import sys
if '/opt/trn_rl_repo' not in sys.path:
    sys.path.insert(0, '/opt/trn_rl_repo')

import numpy as np

N_CORES = 8
L = 512
B_TOT = 4096
U = 8
Y = 16
H = 128
C = U + 1
NI = L - 1            # intervals
HSTEP = 0.25          # dt / SUBSTEPS with dt == 1
B_LOC = B_TOT // N_CORES  # 512

ALPHA = [1.0, 0.1875, -0.25, -0.3125]
BETA = [0.0, 0.8125, 1.25, 1.3125]

_BUILD_CACHE = {}


def _host_constants(W1, b1, W2, b2):
    """Precompute transposed/permuted constant matrices (host-side, free)."""
    rowmap = np.array([(r % 16) * 9 + (r // 16 + 1) for r in range(128)])
    cst = {}
    cst["W1T"] = np.ascontiguousarray(W1.T)                        # (16,128)
    cst["W2aT"] = np.ascontiguousarray(W2[rowmap, :].T)            # (128,128)
    cst["W2bT"] = np.ascontiguousarray(W2[np.arange(16) * 9, :].T)  # (128,16)
    cst["b1c"] = np.ascontiguousarray(b1[:, None])                 # (128,1)
    cst["b2c"] = np.ascontiguousarray(b2[rowmap][:, None])         # (128,1)
    cst["b2t"] = np.ascontiguousarray(b2[np.arange(16) * 9][:, None])  # (16,1)
    # spline eval: dXb[r] = -a*u3[c-1] + (a-b)*u3[8+c-1] + b*u3[16+c-1]
    abc = np.zeros((4, 24, 128), dtype=np.float32)
    for i in range(4):
        for r in range(128):
            c = r // 16 + 1
            abc[i, 0 * 8 + c - 1, r] = -ALPHA[i]
            abc[i, 1 * 8 + c - 1, r] = ALPHA[i] - BETA[i]
            abc[i, 2 * 8 + c - 1, r] = BETA[i]
    cst["Abc"] = abc                                               # (4,24,128)
    # state update matrix: hpre += (h*W1*Sel^T) @ tmp, [r, j] = h*W1[j, r%16]
    w1selt = np.zeros((128, 128), dtype=np.float32)
    for r in range(128):
        w1selt[r, :] = HSTEP * W1[:, r % 16]
    cst["W1SelT"] = w1selt
    return {k: v.astype(np.float32) for k, v in cst.items()}


def _build(n_intervals=NI):
    """Build + compile the Bass module (cached per interval count)."""
    key = n_intervals
    if key in _BUILD_CACHE:
        return _BUILD_CACHE[key]

    import concourse.bass as bass
    import concourse.bacc as bacc
    import concourse.tile as tile
    from concourse import mybir

    F32 = mybir.dt.float32
    TANH = mybir.ActivationFunctionType.Tanh
    MULT = mybir.AluOpType.mult
    ADD = mybir.AluOpType.add

    nc = bacc.Bacc("TRN2", target_bir_lowering=False, debug=False,
                   num_devices=N_CORES)

    d_us3 = nc.dram_tensor("us3", (n_intervals, 24, B_LOC), F32,
                           kind="ExternalInput")
    d_ys0 = nc.dram_tensor("ys0T", (16, B_LOC), F32, kind="ExternalInput")
    d_W1T = nc.dram_tensor("W1T", (16, 128), F32, kind="ExternalInput")
    d_W2aT = nc.dram_tensor("W2aT", (128, 128), F32, kind="ExternalInput")
    d_W2bT = nc.dram_tensor("W2bT", (128, 16), F32, kind="ExternalInput")
    d_b1 = nc.dram_tensor("b1c", (128, 1), F32, kind="ExternalInput")
    d_b2c = nc.dram_tensor("b2c", (128, 1), F32, kind="ExternalInput")
    d_b2t = nc.dram_tensor("b2t", (16, 1), F32, kind="ExternalInput")
    d_Abc = nc.dram_tensor("Abc", (4, 24, 128), F32, kind="ExternalInput")
    d_W1SelT = nc.dram_tensor("W1SelT", (128, 128), F32, kind="ExternalInput")
    d_hout = nc.dram_tensor("hout", (n_intervals, 128, B_LOC), F32,
                            kind="ExternalOutput")

    with tile.TileContext(nc) as tc:
        with (
            tc.tile_pool(name="consts", bufs=1) as consts,
            tc.tile_pool(name="work", bufs=2) as work,
            tc.tile_pool(name="u3p", bufs=4) as u3p,
            tc.tile_pool(name="hsp", bufs=2) as hsp,
            tc.tile_pool(name="ps1", bufs=1, space="PSUM") as ps1,
            tc.tile_pool(name="ps2", bufs=1, space="PSUM") as ps2,
            tc.tile_pool(name="psx", bufs=2, space="PSUM") as psx,
        ):
            W1T = consts.tile([16, 128], F32)
            W2aT = consts.tile([128, 128], F32)
            W2bT = consts.tile([128, 16], F32)
            b1c = consts.tile([128, 1], F32)
            b2c = consts.tile([128, 1], F32)
            b2t = consts.tile([16, 1], F32)
            W1SelT = consts.tile([128, 128], F32)
            Abc = [consts.tile([24, 128], F32, name=f"Abc{i}") for i in range(4)]
            nc.sync.dma_start(W1T[:], d_W1T.ap())
            nc.sync.dma_start(W2aT[:], d_W2aT.ap())
            nc.sync.dma_start(W2bT[:], d_W2bT.ap())
            nc.sync.dma_start(b1c[:], d_b1.ap())
            nc.sync.dma_start(b2c[:], d_b2c.ap())
            nc.sync.dma_start(b2t[:], d_b2t.ap())
            nc.sync.dma_start(W1SelT[:], d_W1SelT.ap())
            for i in range(4):
                nc.sync.dma_start(Abc[i][:], d_Abc.ap()[i])

            z0 = consts.tile([16, B_LOC], F32)
            nc.sync.dma_start(z0[:], d_ys0.ap())

            # hpre is THE state: a persistent PSUM accumulator holding W1 @ z.
            hpre = ps1.tile([128, B_LOC], F32, tag="hpre")
            nc.tensor.matmul(hpre[:], W1T[:], z0[:], start=True, stop=False,
                             skip_group_check=True)

            HB = B_LOC // 2
            COLS = [(0, HB), (HB, B_LOC)]
            u3s = {}

            def load_u3(k):
                if k < n_intervals:
                    t = u3p.tile([24, B_LOC], F32, tag="u3", name=f"u3_{k}")
                    nc.sync.dma_start(t[:], d_us3.ap()[k])
                    u3s[k] = t

            for k in range(3):
                load_u3(k)

            # dXb for substep g, one PSUM tile per half, computed one substep
            # ahead at the PE's two natural stall points.
            dxbs = {}

            def dxb_mm(g, h):
                if g >= 4 * n_intervals:
                    return
                k, i = divmod(g, 4)
                c0, c1 = COLS[h]
                t = psx.tile([128, HB], F32, tag=f"dxb{h}",
                             name=f"dxb{h}_{g}")
                nc.tensor.matmul(t[:], Abc[i][:], u3s[k][:, c0:c1],
                                 start=True, stop=True)
                dxbs[(g, h)] = t

            # substep 0 halves up front
            dxb_mm(0, 0)
            dxb_mm(0, 1)

            for k in range(n_intervals):
                load_u3(k + 3)
                for i in range(4):
                    g = 4 * k + i
                    ha, hb = 0, 1
                    (a0, a1), (b0, b1_) = COLS[ha], COLS[hb]
                    dxa = dxbs.pop((g, ha))
                    dxb = dxbs.pop((g, hb))
                    th = work.tile([128, B_LOC], F32, tag="th")
                    yva = [ps2.tile([128, HB], F32, tag=f"yva{h}",
                                    name=f"yva{h}_{g}") for h in range(2)]
                    yvb = ps2.tile([16, B_LOC], F32, tag="yvb", name=f"yvb_{g}")
                    vfc = work.tile([128, B_LOC], F32, tag="vfc")
                    vft = work.tile([16, B_LOC], F32, tag="vft")
                    tmp = work.tile([128, B_LOC], F32, tag="tmp")

                    # PE queue: dxb(g+1,ha), yva_a, yvb_a, yva_b, yvb_b,
                    #           dxb(g+1,hb), W1Sel_a, W1Sel_b
                    dxb_mm(g + 1, ha)
                    nc.scalar.activation(th[:, a0:a1], hpre[:, a0:a1],
                                         TANH, bias=b1c[:])
                    nc.tensor.matmul(yva[ha][:], W2aT[:], th[:, a0:a1],
                                     start=True, stop=True)
                    nc.scalar.activation(th[:, b0:b1_], hpre[:, b0:b1_],
                                         TANH, bias=b1c[:])
                    nc.tensor.matmul(yvb[:, a0:a1], W2bT[:], th[:, a0:a1],
                                     start=True, stop=True)
                    nc.scalar.activation(vfc[:, a0:a1], yva[ha][:], TANH,
                                         bias=b2c[:])
                    nc.scalar.activation(vft[:, a0:a1], yvb[:, a0:a1],
                                         TANH, bias=b2t[:])
                    nc.vector.tensor_tensor(tmp[:, a0:a1], vfc[:, a0:a1],
                                            dxa[:], MULT)
                    # tmp rows 0..15 (channel 1) also carry the time channel:
                    # + vft adds HSTEP*W1@vft to hpre below.
                    nc.vector.tensor_tensor(tmp[:16, a0:a1], tmp[:16, a0:a1],
                                            vft[:, a0:a1], ADD)
                    nc.tensor.matmul(yva[hb][:], W2aT[:], th[:, b0:b1_],
                                     start=True, stop=True)
                    nc.tensor.matmul(yvb[:, b0:b1_], W2bT[:], th[:, b0:b1_],
                                     start=True, stop=True)
                    nc.scalar.activation(vfc[:, b0:b1_], yva[hb][:], TANH,
                                         bias=b2c[:])
                    nc.scalar.activation(vft[:, b0:b1_], yvb[:, b0:b1_],
                                         TANH, bias=b2t[:])
                    nc.vector.tensor_tensor(tmp[:, b0:b1_], vfc[:, b0:b1_],
                                            dxb[:], MULT)
                    nc.vector.tensor_tensor(tmp[:16, b0:b1_], tmp[:16, b0:b1_],
                                            vft[:, b0:b1_], ADD)
                    dxb_mm(g + 1, hb)
                    nc.tensor.matmul(hpre[:, a0:a1], W1SelT[:],
                                     tmp[:, a0:a1], start=False,
                                     stop=False, skip_group_check=True)
                    nc.tensor.matmul(hpre[:, b0:b1_], W1SelT[:],
                                     tmp[:, b0:b1_], start=False,
                                     stop=False, skip_group_check=True)
                # per-interval output: snapshot hpre in halves (each half as
                # soon as its last W1Sel lands); host recovers
                # z_{k+1} = pinv(W1) @ hpre.
                hps = hsp.tile([128, B_LOC], F32, tag="hps")
                nc.vector.tensor_copy(hps[:, :HB], hpre[:, :HB])
                nc.vector.tensor_copy(hps[:, HB:], hpre[:, HB:])
                nc.sync.dma_start(d_hout.ap()[k], hps[:])

    nc.compile()
    _BUILD_CACHE[key] = nc
    return nc


def _prep_core_inputs(us, ys, cst, core, n_intervals):
    b0 = core * B_LOC
    usc = np.ascontiguousarray(us[:, b0:b0 + B_LOC, :].transpose(0, 2, 1))  # (L,8,B)
    us_ext = np.concatenate([2.0 * usc[:1] - usc[1:2], usc], axis=0)  # (L+1,8,B)
    sw = np.lib.stride_tricks.sliding_window_view(us_ext, 3, axis=0)  # (L-1,8,B,3)
    us3 = np.ascontiguousarray(sw.transpose(0, 3, 1, 2).reshape(L - 1, 24, B_LOC))
    us3 = us3[:n_intervals].astype(np.float32)
    ys0T = np.ascontiguousarray(ys[0, b0:b0 + B_LOC, :].T).astype(np.float32)
    m = {"us3": us3, "ys0T": ys0T}
    m.update(cst)
    return m


def kernel(ts, us, ys, W1, b1, W2, b2, batch_size=None, n_intervals=NI):
    from concourse.bass_utils import run_bass_kernel_spmd

    us = np.asarray(us, dtype=np.float32)
    ys = np.asarray(ys, dtype=np.float32)
    W1 = np.asarray(W1, np.float32)
    cst = _host_constants(W1, np.asarray(b1, np.float32),
                          np.asarray(W2, np.float32), np.asarray(b2, np.float32))
    nc = _build(n_intervals)
    in_maps = [_prep_core_inputs(us, ys, cst, c, n_intervals) for c in range(N_CORES)]
    res = run_bass_kernel_spmd(nc, in_maps, core_ids=list(range(N_CORES)))
    # output reconstruction: z = pinv(W1) @ hpre  (W1 is 128x16, cond ~2)
    R = np.linalg.pinv(W1.astype(np.float64)).astype(np.float32)   # (16,128)
    out = np.empty((B_TOT, n_intervals + 1, Y), dtype=np.float32)
    out[:, 0, :] = ys[0]
    for c in range(N_CORES):
        b0 = c * B_LOC
        hout = res.results[c]["hout"]                # (NI, 128, B_LOC)
        z = np.tensordot(R, hout, axes=(1, 1))       # (16, NI, B_LOC)
        out[b0:b0 + B_LOC, 1:, :] = z.transpose(2, 1, 0)
    kernel._last_results = res
    return out
